# revision 9
# baseline (speedup 1.0000x reference)
"""Bahdanau additive attention kernel for Trainium2 (8 NeuronCores).

Reference computation (B=32, S=4096, D=512):
    pre   = enc @ We.T + (hidden @ Wh.T + b1)[:, None, :]   # [B, S, D]
    h     = tanh(pre)
    e     = h @ w2                                          # [B, S]
    alpha = softmax(e, axis=1)
    ctx   = einsum('bs,bsd->bd', alpha, enc)                # [B, D]

Data-parallel over batch (4 batches per core); enc is re-laid-out on host
as [b, di, p, s] bf16 so the contraction dim d sits on SBUF partitions.

Precision plan (gate is rel-l2 < 2e-2 on ctx):
  - ctx is a softmax-weighted mean of zero-mean vectors - the signal
    shrinks exactly as fast as independent noise averages out, so any
    per-element quantization error lands on ctx at full relative
    strength. Full-fp8 main matmul measures 2.2e-2 (fails); half the
    contraction in fp8 measures ~1.6e-2 (passes).
  - So the main matmul contracts d 0..255 with ONE fp8-e4m3 DoubleRow
    matmul per (ki, s-block) (256-deep contraction per instruction,
    ~0.57x the PE cost of bf16) and d 256..511 with 2 bf16 matmuls.
  - Scale bridge: fp8 operands are scaled (enc x32, W x1024) so their
    PSUM contribution is 2^15 x pre; the bf16 weight half is pre-scaled
    x2^15 on host (exact power of two), and the tanh ACT applies
    scale=2^-15. All contributions share one PSUM accumulation group
    with zero extra ops.
  - Everything downstream of pre stays bf16 (h, p rows, ctx-accum enc).

Schedule (per core):
  - The tiny bias vector c = hidden @ Wh.T + b1 is computed on host
    and rides the ACT bias operand.
  - PE warm-up burst on a memzero'd tile brings the HAM clock gate to
    8/8 while the first enc tile streams in.
  - e matmuls are column-group packed (tile_position=(0,0)/(0,32)) into
    ONE [48, 512] PSUM tile: group 0 uses M=32 (w2 in column 0, zeros
    elsewhere) so PSUM rows 1..31 are written zeros, letting a SINGLE
    exp ACT over partitions [0:33] handle both groups' rows (halves the
    Scalar exp cost vs two [1,512] ACTs; rows 1..31 accumulate exp(0)
    into lp rows that the finalize never reads).
  - Half-tile software pipelining: tile t's e/softmax/context chain is
    emitted after tile t+1's first ki group, so the in-order PE queue
    never waits on tanh.
  - exp runs with fused row-sum accumulation (unnormalized streaming
    softmax: |e| < ~5 so no max pass); per-batch row sums land in lp
    rows 0/32, are reduced along tiles, row 32 is staged down to
    partition 0 by a tiny DMA, summed, reciprocal'd, and broadcast to
    [128,1] for the final context scaling.
  - alpha rows are replicated across partitions by gpsimd
    partition_broadcast COMPUTE ops (no DRAM round trip; broadcast APs
    must be canonical - full-tile dest, partition-0/offset-0 source, so
    the second e row is staged down from partition 32 by a tiny
    scalar-issued copy; sliced/offset APs silently misread on HW).
  - Strict DMA queue roles: sync carries ONLY enc tiles (fp8 pairs tile
    first, then bf16), so the PE-feeding stream is never queued behind
    softmax-dependent writes; gpsimd issues NO DMAs at all.
  - Variable s-tile widths: batch 0 opens with a 512-wide tile (halves
    the pipeline-fill DMA latency), the last batch closes with two
    512-wide tiles (halves the post-PE softmax/context drain).
"""

import sys

if "/opt/trn_rl_repo" not in sys.path:
    sys.path.insert(0, "/opt/trn_rl_repo")

from contextlib import ExitStack

import ml_dtypes
import numpy as np

import concourse.bass as bass
import concourse.bacc as bacc
from concourse import bass_isa
import concourse.tile as tile
from concourse import mybir
from concourse.bass_utils import run_bass_kernel_spmd

B, S, D = 32, 4096, 512
NCORES = 8
BPC = B // NCORES          # batches per core
P = 128                    # partitions
NDC = D // P               # d (contraction) chunks
NKC = D // P               # k (output channel) chunks
ST = 1024                  # max s-tile width (buffer sizing)
EST = 512                  # e-row granularity
SE = 32.0                  # fp8 enc scale
SW = 1024.0                # fp8 weight scale
SINV = 1.0 / (SE * SW)     # 2^-15, ACT tanh scale
# s-tile widths per batch: a narrow first tile on batch 0 shortens the
# pipeline-fill DMA latency; narrow last tiles on the final batch shorten
# the softmax/context drain after the PE finishes.
BATCH_WIDTHS = [
    [512, 1024, 1024, 1024, 512],
    [1024, 1024, 1024, 1024],
    [1024, 1024, 1024, 1024],
    [1024, 1024, 1024, 512, 256, 256],
]
NST = max(len(w) for w in BATCH_WIDTHS)

F32 = mybir.dt.float32
BF16 = mybir.dt.bfloat16
FP8 = mybir.dt.float8e4
AF = mybir.ActivationFunctionType
ALU = mybir.AluOpType
DR = mybir.MatmulPerfMode.DoubleRow


def build_bass():
    nc = bacc.Bacc()

    encT = nc.declare_dram_parameter("encT", [BPC, NDC, P, S], BF16, isOutput=False)
    enc8T = nc.declare_dram_parameter("enc8T", [BPC, P, 2, S], FP8, isOutput=False)
    # weT[i, p, k] = We[k, 256 + i*128 + p] * 2^15  (bf16 half, pre-scaled)
    weT = nc.declare_dram_parameter("weT", [2, P, D], BF16, isOutput=False)
    # w8T[p, j, k] = We[k, j*128 + p] * 1024  (fp8 DoubleRow half)
    w8T = nc.declare_dram_parameter("w8T", [P, 2, D], FP8, isOutput=False)
    cT = nc.declare_dram_parameter("cT", [P, NKC, BPC], F32, isOutput=False)
    # w2c[p, ki, m]: column 0 holds w2[ki*128+p], columns 1..31 zero
    w2c = nc.declare_dram_parameter("w2c", [P, NKC, 32], BF16, isOutput=False)
    ctx_out = nc.declare_dram_parameter("ctx", [P, NDC, BPC], F32, isOutput=True)

    with TileKernel(nc) as tk:
        tk.build(encT, enc8T, weT, w8T, cT, w2c, ctx_out)
    nc.finalize()
    return nc


class TileKernel:
    def __init__(self, nc):
        self.nc = nc
        self.stack = ExitStack()
        self.tc = None

    def __enter__(self):
        self.tc = self.stack.enter_context(tile.TileContext(self.nc))
        return self

    def __exit__(self, *exc):
        return self.stack.__exit__(*exc)

    def build(self, encT, enc8T, weT, w8T, cT, w2c, ctx_out):
        nc, tc, ctx = self.nc, self.tc, self.stack

        singles = ctx.enter_context(tc.tile_pool(name="singles", bufs=1))
        encp = ctx.enter_context(tc.tile_pool(name="encp", bufs=8))
        encp8 = ctx.enter_context(tc.tile_pool(name="encp8", bufs=8))
        htp = ctx.enter_context(tc.tile_pool(name="htp", bufs=4))
        abp = ctx.enter_context(tc.tile_pool(name="abp", bufs=8))
        junkp = ctx.enter_context(tc.tile_pool(name="junkp", bufs=3))
        smp = ctx.enter_context(tc.tile_pool(name="smp", bufs=3))
        ctxp = ctx.enter_context(tc.tile_pool(name="ctxp", bufs=2))
        prep = ctx.enter_context(tc.tile_pool(name="prep", bufs=3, space="PSUM"))
        ecp = ctx.enter_context(tc.tile_pool(name="ecp", bufs=2, space="PSUM"))

        # ---- load constants ----
        # constants go via the scalar queue so they don't delay enc tile 0
        # on the sync queue.
        w8_sb = singles.tile([P, 2, D], FP8)
        nc.scalar.dma_start(out=w8_sb, in_=w8T[:])
        w_sb = singles.tile([P, 2, D], BF16)
        nc.scalar.dma_start(out=w_sb, in_=weT[:].rearrange("i p k -> p i k"))
        w2_sb = singles.tile([P, NKC, 32], BF16)
        nc.scalar.dma_start(out=w2_sb, in_=w2c[:])
        c_sb = singles.tile([P, NKC, BPC], F32)
        nc.scalar.dma_start(out=c_sb, in_=cT[:])

        # ---- PE warm-up burst ----
        # ~2.3 us of dummy matmuls on a zeroed tile (no DMA dependency) so
        # the HAM clock gate reaches 8/8 before real work starts.
        wz = singles.tile([P, D], BF16)
        nc.vector.memzero(wz)
        wpre = prep.tile([P, ST], F32, tag="pre")
        for i in range(11):
            nc.tensor.matmul(
                out=wpre[:, 0:D], lhsT=wz[:, 0:P], rhs=wz,
                start=True, stop=True,
            )
        wjunk = singles.tile([P, 1], F32)
        nc.vector.tensor_copy(out=wjunk, in_=wpre[:, 0:1])

        # ---- main per-batch pipeline ----
        # `prev` carries one pending softmax/context chain ACROSS batch
        # boundaries (flushed inside the next tile's first ki group), so
        # the PE never drains at a batch edge; each chain closes over its
        # own batch's lp/cacc tiles.
        prev = None
        pend_fin = None

        def softmax_ctx(t, w, et, ht, lp, cacc, is_last):
            ng = (w + EST - 1) // EST
                # e rows, packed on PE column groups 0/32 into one PSUM
                # tile: the (up to) two accumulation chains run
                # concurrently (separate XBUS streams, separate PSUM
                # partition ranges). Group 0 uses M=32 so rows 1..31 are
                # written zeros - that makes the single [0:33] exp ACT
                # read only initialized PSUM.
            # The two chains are separate accumulation groups over
            # disjoint partition ranges of one tile; pending-zero is
            # tracked per out-AP partitions, so each chain needs its
            # own start. skip_group_check silences the coarser
            # tile-framework zero-region conflict check for chain 1.
            w0 = min(w, EST)
            w1 = w - w0
            e_ps = ecp.tile([48, EST], F32, tag="ec")
            for ki in range(NKC):
                nc.tensor.matmul(
                    out=e_ps[0:32, 0:w0],
                    lhsT=w2_sb[:, ki, :],
                    rhs=ht[:, ki, 0:w0],
                    start=(ki == 0),
                    stop=(ki == NKC - 1),
                    tile_position=(0, 0),
                )
                if ng == 2:
                    nc.tensor.matmul(
                        out=e_ps[32:48, 0:w1],
                        lhsT=w2_sb[:, ki, 0:16],
                        rhs=ht[:, ki, EST:EST + w1],
                        start=(ki == 0),
                        stop=(ki == NKC - 1),
                        tile_position=(0, 32),
                        skip_group_check=True,
                    )
            # p = exp(e) with the row-sum fused; one ACT covers both
            # groups' live rows (partitions 0 and 32). Merging requires
            # both groups full-width.
            p_rows = smp.tile([48, EST], BF16, tag="prow")
            np_act = 33 if (ng == 2 and w1 == EST) else 1
            nc.scalar.activation(
                out=p_rows[0:np_act, 0:w0],
                in_=e_ps[0:np_act, 0:w0],
                func=AF.Exp, bias=0.0, scale=1.0,
                accum_out=lp[0:np_act, t:t + 1],
            )
            if ng == 2 and np_act == 1:
                nc.scalar.activation(
                    out=p_rows[32:33, 0:w1],
                    in_=e_ps[32:33, 0:w1],
                    func=AF.Exp, bias=0.0, scale=1.0,
                    accum_out=lp[32:33, t:t + 1],
                )
            if is_last:
                # early lsum extraction: the exp row sums are complete
                # once this (final) chain's exp has run; start the
                # cross-partition staging DMA now so its latency hides
                # under the context accumulation below.
                lpr = smp.tile([48, 1], F32, tag="lpr")
                nc.vector.reduce_sum(out=lpr, in_=lp, axis=mybir.AxisListType.X)
                ls1 = smp.tile([1, 1], F32, tag="ls1")
                nc.scalar.dma_start(out=ls1, in_=lpr[32:33, :])
                self._fin = (lpr, ls1)
            # replicate the p rows across partitions with gpsimd
            # partition_broadcast compute ops: no DRAM round trip, and
            # gpsimd stays DMA-free. The broadcast gets canonical APs
            # only: full-tile outputs, and a partition-0/offset-0
            # source (row g=1 is staged down from partition 32 by a
            # tiny scalar-issued SBUF->SBUF copy).
            ab = abp.tile([P, ST], BF16, tag="ab")
            for g in range(ng):
                if g == 0:
                    src_row = p_rows[0:1, 0:w0]
                    dst = ab[:, 0:w0]
                else:
                    pr1 = smp.tile([1, EST], BF16, tag="pr1")
                    nc.scalar.dma_start(out=pr1[:, 0:w1], in_=p_rows[32:33, 0:w1])
                    src_row = pr1[:, 0:w1]
                    dst = ab[:, EST:EST + w1]
                nc.gpsimd.partition_broadcast(out_ap=dst, in_ap=src_row)
            for di in (2, 3, 0, 1):
                junk = junkp.tile([P, ST], BF16, tag="junk")
                nc.vector.scalar_tensor_tensor(
                    out=junk[:, 0:w],
                    in0=et[:, di, 0:w],
                    scalar=1.0,
                    in1=ab[:, 0:w],
                    op0=ALU.mult,
                    op1=ALU.mult,
                    accum_out=cacc[:, di, t:t + 1],
                )

        def finalize(b, lp, cacc):
            # ---- finalize: ctx = (sum_s p*enc) / sum_s p ----
            # lp rows 0 and 32 hold the two groups' per-tile exp sums
            # (rows 1..31 hold junk exp(0) sums the finalize never
            # reads); the reduce + row-32 staging DMA were emitted with
            # the batch's last chain.
            lpr, ls1 = self._fin
            lsum = smp.tile([1, 1], F32, tag="lsum")
            nc.vector.tensor_add(out=lsum, in0=lpr[0:1, :], in1=ls1)
            rinv1 = smp.tile([1, 1], F32, tag="rinv1")
            nc.vector.reciprocal(out=rinv1, in_=lsum)
            rinvb = smp.tile([P, 1], F32, tag="rinvb")
            nc.gpsimd.partition_broadcast(out_ap=rinvb, in_ap=rinv1)
            ctx_acc = ctxp.tile([P, NDC], F32, tag="ctx")
            nc.vector.reduce_sum(out=ctx_acc, in_=cacc, axis=mybir.AxisListType.X)
            nc.vector.tensor_scalar_mul(out=ctx_acc, in0=ctx_acc, scalar1=rinvb)
            nc.scalar.dma_start(out=ctx_out[:][:, :, b], in_=ctx_acc)

        for b in range(BPC):
            nbt = len(BATCH_WIDTHS[b])
            lp = smp.tile([48, NST], F32, tag="lp")
            nc.vector.memzero(lp)
            cacc = ctxp.tile([P, NDC, nbt], F32, tag="cacc")

            s0 = 0
            for t, w in enumerate(BATCH_WIDTHS[b]):
                et8 = encp8.tile([P, 2, ST], FP8, tag="et8")
                nc.sync.dma_start(
                    out=et8[:, :, 0:w],
                    in_=enc8T[:][b, :, :, s0:s0 + w])
                et = encp.tile([P, NDC, ST], BF16, tag="et")
                nc.sync.dma_start(
                    out=et[:, 2:4, 0:w],
                    in_=encT[:][b, 2:4, :, s0:s0 + w].rearrange("di p s -> p di s"))
                nc.scalar.dma_start(
                    out=et[:, 0:2, 0:w],
                    in_=encT[:][b, 0:2, :, s0:s0 + w].rearrange("di p s -> p di s"))

                ht = htp.tile([P, NKC, ST], BF16, tag="ht")
                for ki in range(NKC):
                    pre_ps = prep.tile([P, ST], F32, tag="pre")
                    for half in range((w + EST - 1) // EST):
                        hw_ = min(EST, w - half * EST)
                        sl = slice(half * EST, half * EST + hw_)
                        # d 0..255: one fp8 DoubleRow matmul (256-deep)
                        nc.tensor.matmul(
                            out=pre_ps[:, sl],
                            lhsT=w8_sb[:, :, ki * P:(ki + 1) * P],
                            rhs=et8[:, :, sl],
                            start=True,
                            stop=False,
                            perf_mode=DR,
                        )
                        # d 256..511: two bf16 matmuls (weights x 2^15)
                        for i in range(2):
                            nc.tensor.matmul(
                                out=pre_ps[:, sl],
                                lhsT=w_sb[:, i, ki * P:(ki + 1) * P],
                                rhs=et[:, 2 + i, sl],
                                start=False,
                                stop=(i == 1),
                            )
                    # h^T = tanh(2^-15 * pre^T + c), one ACT op per ki
                    nc.scalar.activation(
                        out=ht[:, ki, 0:w],
                        in_=pre_ps[:, 0:w],
                        func=AF.Tanh,
                        bias=c_sb[:, ki, b:b + 1],
                        scale=SINV,
                    )
                    # half-tile software pipelining: the previous tile's
                    # e/softmax/context chain slots in after this tile's
                    # first ki group, when its tanh inputs are ready but
                    # well before the PE would stall on them. Chains (and
                    # the previous batch's finalize) cross batch edges.
                    if ki == 1 and prev is not None:
                        softmax_ctx(*prev)
                        prev = None
                        if pend_fin is not None:
                            finalize(*pend_fin)
                            pend_fin = None
                prev = (t, w, et, ht, lp, cacc, False)
                s0 += w
            prev = prev[:6] + (True,)
            pend_fin = (b, lp, cacc)

        softmax_ctx(*prev)
        finalize(*pend_fin)


_NC_CACHE = None


def _get_nc():
    global _NC_CACHE
    if _NC_CACHE is None:
        _NC_CACHE = build_bass()
    return _NC_CACHE


def _prep_core_inputs(hidden_state, encoder_outputs, W1, b1, w2, core):
    bf16 = ml_dtypes.bfloat16
    e4m3 = ml_dtypes.float8_e4m3
    b0 = core * BPC
    enc = encoder_outputs[b0:b0 + BPC]                      # [BPC, S, D] f32
    # [b, d, s] -> [b, di, p, s] flat along s
    e = enc.transpose(0, 2, 1).reshape(BPC, NDC, P, S).astype(bf16)
    e = np.ascontiguousarray(e)
    # fp8 pairs for d 0..255: enc8[b, p, j, s] = e4m3(32 * enc[b, s, j*128+p])
    e8 = np.clip(enc.transpose(0, 2, 1)[:, 0:256, :] * SE, -240.0, 240.0)
    e8 = np.ascontiguousarray(
        e8.reshape(BPC, 2, P, S).transpose(0, 2, 1, 3)).astype(e4m3)
    # bf16 weight half (d 256..511), pre-scaled by 2^15
    weS = (W1[:, 256:D].T * (SE * SW)).reshape(2, P, D)
    # fp8 weight half (d 0..255): w8[p, j, k] = e4m3(1024 * We[k, j*128+p])
    w8 = np.clip(W1[:, 0:256].T * SW, -240.0, 240.0).reshape(2, P, D)
    w8 = np.ascontiguousarray(w8.transpose(1, 0, 2)).astype(e4m3)
    w2cv = np.zeros((P, NKC, 32), dtype=np.float32)
    w2cv[:, :, 0] = w2.reshape(NKC, P).T
    return {
        "encT": e,
        "enc8T": e8,
        "weT": np.ascontiguousarray(weS).astype(bf16),
        "w8T": w8,
        "cT": np.ascontiguousarray(
            (hidden_state[b0:b0 + BPC] @ W1[:, D:].T + b1).T.reshape(NKC, P, BPC)
            .transpose(1, 0, 2)),
        "w2c": w2cv.astype(bf16),
    }


def kernel(hidden_state, encoder_outputs, W1, b1, w2, _trace=False, _trace_kwargs=None):
    hidden_state = np.asarray(hidden_state, dtype=np.float32)
    encoder_outputs = np.asarray(encoder_outputs, dtype=np.float32)
    W1 = np.asarray(W1, dtype=np.float32)
    b1 = np.asarray(b1, dtype=np.float32)
    w2 = np.asarray(w2, dtype=np.float32)

    nc = _get_nc()
    in_maps = [
        _prep_core_inputs(hidden_state, encoder_outputs, W1, b1, w2, c)
        for c in range(NCORES)
    ]
    res = run_bass_kernel_spmd(
        nc, in_maps, list(range(NCORES)), trace=_trace,
        **(_trace_kwargs or {}),
    )
    out = np.empty((B, D), dtype=np.float32)
    for c in range(NCORES):
        r = res.results[c]["ctx"]                          # [p, di, b]
        out[c * BPC:(c + 1) * BPC] = r.transpose(2, 1, 0).reshape(BPC, D)
    if _trace:
        return out, res
    return out


# revision 10
# speedup vs baseline: 1.0334x; 1.0334x over previous
"""Bahdanau additive attention kernel for Trainium2 (8 NeuronCores).

Reference computation (B=32, S=4096, D=512):
    pre   = enc @ We.T + (hidden @ Wh.T + b1)[:, None, :]   # [B, S, D]
    h     = tanh(pre)
    e     = h @ w2                                          # [B, S]
    alpha = softmax(e, axis=1)
    ctx   = einsum('bs,bsd->bd', alpha, enc)                # [B, D]

Data-parallel over batch (4 batches per core); enc is re-laid-out on host
as [b, di, p, s] bf16 so the contraction dim d sits on SBUF partitions.

Precision plan (gate is rel-l2 < 2e-2 on ctx):
  - ctx is a softmax-weighted mean of zero-mean vectors - the signal
    shrinks exactly as fast as independent noise averages out, so any
    per-element quantization error lands on ctx at full relative
    strength. Full-fp8 main matmul measures 2.2e-2 (fails); half the
    contraction in fp8 measures ~1.6e-2 (passes).
  - So the main matmul contracts d 0..255 with ONE fp8-e4m3 DoubleRow
    matmul per (ki, s-block) (256-deep contraction per instruction,
    ~0.57x the PE cost of bf16) and d 256..511 with 2 bf16 matmuls.
  - Scale bridge: fp8 operands are scaled (enc x32, W x1024) so their
    PSUM contribution is 2^15 x pre; the bf16 weight half is pre-scaled
    x2^15 on host (exact power of two), and the tanh ACT applies
    scale=2^-15. All contributions share one PSUM accumulation group
    with zero extra ops.
  - Everything downstream of pre stays bf16 (h, p rows, ctx-accum enc).

Schedule (per core):
  - The tiny bias vector c = hidden @ Wh.T + b1 is computed on host
    and rides the ACT bias operand.
  - PE warm-up burst on a memzero'd tile brings the HAM clock gate to
    8/8 while the first enc tile streams in.
  - e matmuls are column-group packed (tile_position=(0,0)/(0,32)) into
    ONE [48, 512] PSUM tile: group 0 uses M=32 (w2 in column 0, zeros
    elsewhere) so PSUM rows 1..31 are written zeros, letting a SINGLE
    exp ACT over partitions [0:33] handle both groups' rows (halves the
    Scalar exp cost vs two [1,512] ACTs; rows 1..31 accumulate exp(0)
    into lp rows that the finalize never reads).
  - Half-tile software pipelining: tile t's e/softmax/context chain is
    emitted after tile t+1's first ki group, so the in-order PE queue
    never waits on tanh.
  - exp runs with fused row-sum accumulation (unnormalized streaming
    softmax: |e| < ~5 so no max pass); per-batch row sums land in lp
    rows 0/32, are reduced along tiles, row 32 is staged down to
    partition 0 by a tiny DMA, summed, reciprocal'd, and broadcast to
    [128,1] for the final context scaling.
  - alpha rows are replicated across partitions by gpsimd
    partition_broadcast COMPUTE ops (no DRAM round trip; broadcast APs
    must be canonical - full-tile dest, partition-0/offset-0 source, so
    the second e row is staged down from partition 32 by a tiny
    scalar-issued copy; sliced/offset APs silently misread on HW).
  - Strict DMA queue roles: sync carries ONLY enc tiles (fp8 pairs tile
    first, then bf16), so the PE-feeding stream is never queued behind
    softmax-dependent writes; gpsimd issues NO DMAs at all.
  - Variable s-tile widths: batch 0 opens with a 512-wide tile (halves
    the pipeline-fill DMA latency), the last batch closes with two
    512-wide tiles (halves the post-PE softmax/context drain).
"""

import sys

if "/opt/trn_rl_repo" not in sys.path:
    sys.path.insert(0, "/opt/trn_rl_repo")

from contextlib import ExitStack

import ml_dtypes
import numpy as np

import concourse.bass as bass
import concourse.bacc as bacc
from concourse import bass_isa
import concourse.tile as tile
from concourse import mybir
from concourse.bass_utils import run_bass_kernel_spmd

B, S, D = 32, 4096, 512
NCORES = 8
BPC = B // NCORES          # batches per core
P = 128                    # partitions
NDC = D // P               # d (contraction) chunks
NKC = D // P               # k (output channel) chunks
ST = 1024                  # max s-tile width (buffer sizing)
EST = 512                  # e-row granularity
SE = 32.0                  # fp8 enc scale
SW = 1024.0                # fp8 weight scale
SINV = 1.0 / (SE * SW)     # 2^-15, ACT tanh scale
# s-tile widths per batch: a narrow first tile on batch 0 shortens the
# pipeline-fill DMA latency; narrow last tiles on the final batch shorten
# the softmax/context drain after the PE finishes.
BATCH_WIDTHS = [
    [512, 1024, 1024, 1024, 512],
    [1024, 1024, 1024, 1024],
    [1024, 1024, 1024, 1024],
    [1024, 1024, 1024, 512, 256, 256],
]
NST = max(len(w) for w in BATCH_WIDTHS)

F32 = mybir.dt.float32
BF16 = mybir.dt.bfloat16
FP8 = mybir.dt.float8e4
AF = mybir.ActivationFunctionType
ALU = mybir.AluOpType
DR = mybir.MatmulPerfMode.DoubleRow


def build_bass():
    nc = bacc.Bacc()

    encT = nc.declare_dram_parameter("encT", [BPC, NDC, P, S], BF16, isOutput=False)
    enc8T = nc.declare_dram_parameter("enc8T", [BPC, P, 2, S], FP8, isOutput=False)
    # weT[i, p, k] = We[k, 256 + i*128 + p] * 2^15  (bf16 half, pre-scaled)
    weT = nc.declare_dram_parameter("weT", [2, P, D], BF16, isOutput=False)
    # w8T[p, j, k] = We[k, j*128 + p] * 1024  (fp8 DoubleRow half)
    w8T = nc.declare_dram_parameter("w8T", [P, 2, D], FP8, isOutput=False)
    cT = nc.declare_dram_parameter("cT", [P, NKC, BPC], F32, isOutput=False)
    # w2c[p, ki, m]: column 0 holds w2[ki*128+p], columns 1..31 zero
    w2c = nc.declare_dram_parameter("w2c", [P, NKC, 32], BF16, isOutput=False)
    ctx_out = nc.declare_dram_parameter("ctx", [P, NDC, BPC], F32, isOutput=True)

    with TileKernel(nc) as tk:
        tk.build(encT, enc8T, weT, w8T, cT, w2c, ctx_out)
    nc.finalize()
    return nc


class TileKernel:
    def __init__(self, nc):
        self.nc = nc
        self.stack = ExitStack()
        self.tc = None

    def __enter__(self):
        self.tc = self.stack.enter_context(tile.TileContext(self.nc))
        return self

    def __exit__(self, *exc):
        return self.stack.__exit__(*exc)

    def build(self, encT, enc8T, weT, w8T, cT, w2c, ctx_out):
        nc, tc, ctx = self.nc, self.tc, self.stack

        singles = ctx.enter_context(tc.tile_pool(name="singles", bufs=1))
        encp = ctx.enter_context(tc.tile_pool(name="encp", bufs=8))
        encp8 = ctx.enter_context(tc.tile_pool(name="encp8", bufs=8))
        htp = ctx.enter_context(tc.tile_pool(name="htp", bufs=4))
        abp = ctx.enter_context(tc.tile_pool(name="abp", bufs=8))
        junkp = ctx.enter_context(tc.tile_pool(name="junkp", bufs=3))
        smp = ctx.enter_context(tc.tile_pool(name="smp", bufs=3))
        ctxp = ctx.enter_context(tc.tile_pool(name="ctxp", bufs=2))
        prep = ctx.enter_context(tc.tile_pool(name="prep", bufs=3, space="PSUM"))
        ecp = ctx.enter_context(tc.tile_pool(name="ecp", bufs=2, space="PSUM"))

        # ---- load constants ----
        # constants go via the scalar queue so they don't delay enc tile 0
        # on the sync queue.
        c_sb = singles.tile([P, NKC, BPC], F32)
        nc.scalar.dma_start(out=c_sb, in_=cT[:])
        w8_sb = singles.tile([P, 2, D], FP8)
        nc.scalar.dma_start(out=w8_sb, in_=w8T[:])
        w_sb = singles.tile([P, 2, D], BF16)
        nc.scalar.dma_start(out=w_sb, in_=weT[:].rearrange("i p k -> p i k"))
        w2_sb = singles.tile([P, NKC, 32], BF16)
        nc.scalar.dma_start(out=w2_sb, in_=w2c[:])
        # ---- PE warm-up burst ----
        # ~2.3 us of dummy matmuls on a zeroed tile (no DMA dependency) so
        # the HAM clock gate reaches 8/8 before real work starts.
        wz = singles.tile([P, D], BF16)
        nc.vector.memzero(wz)
        wpre = prep.tile([P, ST], F32, tag="pre")
        for i in range(11):
            nc.tensor.matmul(
                out=wpre[:, 0:D], lhsT=wz[:, 0:P], rhs=wz,
                start=True, stop=True,
            )
        wjunk = singles.tile([P, 1], F32)
        nc.vector.tensor_copy(out=wjunk, in_=wpre[:, 0:1])

        # ---- main per-batch pipeline ----
        # `prev` carries one pending softmax/context chain ACROSS batch
        # boundaries (flushed inside the next tile's first ki group), so
        # the PE never drains at a batch edge; each chain closes over its
        # own batch's lp/cacc tiles.
        prev = None
        pend_fin = None

        def softmax_ctx(t, w, et, ht, lp, cacc, is_last):
            ng = (w + EST - 1) // EST
                # e rows, packed on PE column groups 0/32 into one PSUM
                # tile: the (up to) two accumulation chains run
                # concurrently (separate XBUS streams, separate PSUM
                # partition ranges). Group 0 uses M=32 so rows 1..31 are
                # written zeros - that makes the single [0:33] exp ACT
                # read only initialized PSUM.
            # The two chains are separate accumulation groups over
            # disjoint partition ranges of one tile; pending-zero is
            # tracked per out-AP partitions, so each chain needs its
            # own start. skip_group_check silences the coarser
            # tile-framework zero-region conflict check for chain 1.
            w0 = min(w, EST)
            w1 = w - w0
            e_ps = ecp.tile([48, EST], F32, tag="ec")
            for ki in range(NKC):
                nc.tensor.matmul(
                    out=e_ps[0:32, 0:w0],
                    lhsT=w2_sb[:, ki, :],
                    rhs=ht[:, ki, 0:w0],
                    start=(ki == 0),
                    stop=(ki == NKC - 1),
                    tile_position=(0, 0),
                )
                if ng == 2:
                    nc.tensor.matmul(
                        out=e_ps[32:48, 0:w1],
                        lhsT=w2_sb[:, ki, 0:16],
                        rhs=ht[:, ki, EST:EST + w1],
                        start=(ki == 0),
                        stop=(ki == NKC - 1),
                        tile_position=(0, 32),
                        skip_group_check=True,
                    )
            # p = exp(e) with the row-sum fused; one ACT covers both
            # groups' live rows (partitions 0 and 32). Merging requires
            # both groups full-width.
            p_rows = smp.tile([48, EST], BF16, tag="prow")
            np_act = 33 if (ng == 2 and w1 == EST) else 1
            nc.scalar.activation(
                out=p_rows[0:np_act, 0:w0],
                in_=e_ps[0:np_act, 0:w0],
                func=AF.Exp, bias=0.0, scale=1.0,
                accum_out=lp[0:np_act, t:t + 1],
            )
            if ng == 2 and np_act == 1:
                nc.scalar.activation(
                    out=p_rows[32:33, 0:w1],
                    in_=e_ps[32:33, 0:w1],
                    func=AF.Exp, bias=0.0, scale=1.0,
                    accum_out=lp[32:33, t:t + 1],
                )
            if is_last:
                # early lsum extraction: the exp row sums are complete
                # once this (final) chain's exp has run; start the
                # cross-partition staging DMA now so its latency hides
                # under the context accumulation below.
                lpr = smp.tile([48, 1], F32, tag="lpr")
                nc.vector.reduce_sum(out=lpr, in_=lp, axis=mybir.AxisListType.X)
                ls1 = smp.tile([1, 1], F32, tag="ls1")
                nc.scalar.dma_start(out=ls1, in_=lpr[32:33, :])
                self._fin = (lpr, ls1)
            # replicate the p rows across partitions with gpsimd
            # partition_broadcast compute ops: no DRAM round trip, and
            # gpsimd stays DMA-free. The broadcast gets canonical APs
            # only: full-tile outputs, and a partition-0/offset-0
            # source (row g=1 is staged down from partition 32 by a
            # tiny scalar-issued SBUF->SBUF copy).
            ab = abp.tile([P, ST], BF16, tag="ab")
            for g in range(ng):
                if g == 0:
                    src_row = p_rows[0:1, 0:w0]
                    dst = ab[:, 0:w0]
                else:
                    pr1 = smp.tile([1, EST], BF16, tag="pr1")
                    nc.scalar.dma_start(out=pr1[:, 0:w1], in_=p_rows[32:33, 0:w1])
                    src_row = pr1[:, 0:w1]
                    dst = ab[:, EST:EST + w1]
                nc.gpsimd.partition_broadcast(out_ap=dst, in_ap=src_row)
            for di in (2, 3, 0, 1):
                junk = junkp.tile([P, ST], BF16, tag="junk")
                nc.vector.scalar_tensor_tensor(
                    out=junk[:, 0:w],
                    in0=et[:, di, 0:w],
                    scalar=1.0,
                    in1=ab[:, 0:w],
                    op0=ALU.mult,
                    op1=ALU.mult,
                    accum_out=cacc[:, di, t:t + 1],
                )

        def finalize(b, lp, cacc):
            # ---- finalize: ctx = (sum_s p*enc) / sum_s p ----
            # lp rows 0 and 32 hold the two groups' per-tile exp sums
            # (rows 1..31 hold junk exp(0) sums the finalize never
            # reads); the reduce + row-32 staging DMA were emitted with
            # the batch's last chain.
            lpr, ls1 = self._fin
            lsum = smp.tile([1, 1], F32, tag="lsum")
            nc.vector.tensor_add(out=lsum, in0=lpr[0:1, :], in1=ls1)
            rinv1 = smp.tile([1, 1], F32, tag="rinv1")
            nc.vector.reciprocal(out=rinv1, in_=lsum)
            rinvb = smp.tile([P, 1], F32, tag="rinvb")
            nc.gpsimd.partition_broadcast(out_ap=rinvb, in_ap=rinv1)
            ctx_acc = ctxp.tile([P, NDC], F32, tag="ctx")
            nc.vector.reduce_sum(out=ctx_acc, in_=cacc, axis=mybir.AxisListType.X)
            nc.vector.tensor_scalar_mul(out=ctx_acc, in0=ctx_acc, scalar1=rinvb)
            nc.scalar.dma_start(out=ctx_out[:][:, :, b], in_=ctx_acc)

        for b in range(BPC):
            nbt = len(BATCH_WIDTHS[b])
            lp = smp.tile([48, NST], F32, tag="lp")
            nc.vector.memzero(lp)
            cacc = ctxp.tile([P, NDC, nbt], F32, tag="cacc")

            s0 = 0
            for t, w in enumerate(BATCH_WIDTHS[b]):
                et8 = encp8.tile([P, 2, ST], FP8, tag="et8")
                nc.sync.dma_start(
                    out=et8[:, :, 0:w],
                    in_=enc8T[:][b, :, :, s0:s0 + w])
                et = encp.tile([P, NDC, ST], BF16, tag="et")
                nc.sync.dma_start(
                    out=et[:, :, 0:w],
                    in_=encT[:][b, :, :, s0:s0 + w].rearrange("di p s -> p di s"))

                ht = htp.tile([P, NKC, ST], BF16, tag="ht")
                shift_tanh = (b == 0 and t == 0)
                pend_tanh = []
                for ki in range(NKC):
                    pre_ps = prep.tile([P, ST], F32, tag="pre")
                    for half in range((w + EST - 1) // EST):
                        hw_ = min(EST, w - half * EST)
                        sl = slice(half * EST, half * EST + hw_)
                        # d 0..255: one fp8 DoubleRow matmul (256-deep)
                        nc.tensor.matmul(
                            out=pre_ps[:, sl],
                            lhsT=w8_sb[:, :, ki * P:(ki + 1) * P],
                            rhs=et8[:, :, sl],
                            start=True,
                            stop=False,
                            perf_mode=DR,
                        )
                        # d 256..511: two bf16 matmuls (weights x 2^15)
                        for i in range(2):
                            nc.tensor.matmul(
                                out=pre_ps[:, sl],
                                lhsT=w_sb[:, i, ki * P:(ki + 1) * P],
                                rhs=et[:, 2 + i, sl],
                                start=False,
                                stop=(i == 1),
                            )
                    # h^T = tanh(2^-15 * pre^T + c), one ACT op per ki.
                    # On the first tile the emission is shifted one ki
                    # later so the pipe-fill matmuls never wait on it.
                    def emit_tanh(ki, pre_ps):
                        nc.scalar.activation(
                            out=ht[:, ki, 0:w],
                            in_=pre_ps[:, 0:w],
                            func=AF.Tanh,
                            bias=c_sb[:, ki, b:b + 1],
                            scale=SINV,
                        )
                    if shift_tanh:
                        pend_tanh.append((ki, pre_ps))
                        if len(pend_tanh) > 1:
                            emit_tanh(*pend_tanh.pop(0))
                    else:
                        emit_tanh(ki, pre_ps)
                    # half-tile software pipelining: the previous tile's
                    # e/softmax/context chain slots in after this tile's
                    # first ki group, when its tanh inputs are ready but
                    # well before the PE would stall on them. Chains (and
                    # the previous batch's finalize) cross batch edges.
                    if ki == 1 and prev is not None:
                        softmax_ctx(*prev)
                        prev = None
                        if pend_fin is not None:
                            finalize(*pend_fin)
                            pend_fin = None
                for args_ in pend_tanh:
                    emit_tanh(*args_)
                prev = (t, w, et, ht, lp, cacc, False)
                s0 += w
            prev = prev[:6] + (True,)
            pend_fin = (b, lp, cacc)

        softmax_ctx(*prev)
        finalize(*pend_fin)


_NC_CACHE = None


def _get_nc():
    global _NC_CACHE
    if _NC_CACHE is None:
        _NC_CACHE = build_bass()
    return _NC_CACHE


def _prep_core_inputs(hidden_state, encoder_outputs, W1, b1, w2, core):
    bf16 = ml_dtypes.bfloat16
    e4m3 = ml_dtypes.float8_e4m3
    b0 = core * BPC
    enc = encoder_outputs[b0:b0 + BPC]                      # [BPC, S, D] f32
    # [b, d, s] -> [b, di, p, s] flat along s
    e = enc.transpose(0, 2, 1).reshape(BPC, NDC, P, S).astype(bf16)
    e = np.ascontiguousarray(e)
    # fp8 pairs for d 0..255: enc8[b, p, j, s] = e4m3(32 * enc[b, s, j*128+p])
    e8 = np.clip(enc.transpose(0, 2, 1)[:, 0:256, :] * SE, -240.0, 240.0)
    e8 = np.ascontiguousarray(
        e8.reshape(BPC, 2, P, S).transpose(0, 2, 1, 3)).astype(e4m3)
    # bf16 weight half (d 256..511), pre-scaled by 2^15
    weS = (W1[:, 256:D].T * (SE * SW)).reshape(2, P, D)
    # fp8 weight half (d 0..255): w8[p, j, k] = e4m3(1024 * We[k, j*128+p])
    w8 = np.clip(W1[:, 0:256].T * SW, -240.0, 240.0).reshape(2, P, D)
    w8 = np.ascontiguousarray(w8.transpose(1, 0, 2)).astype(e4m3)
    w2cv = np.zeros((P, NKC, 32), dtype=np.float32)
    w2cv[:, :, 0] = w2.reshape(NKC, P).T
    return {
        "encT": e,
        "enc8T": e8,
        "weT": np.ascontiguousarray(weS).astype(bf16),
        "w8T": w8,
        "cT": np.ascontiguousarray(
            (hidden_state[b0:b0 + BPC] @ W1[:, D:].T + b1).T.reshape(NKC, P, BPC)
            .transpose(1, 0, 2)),
        "w2c": w2cv.astype(bf16),
    }


def kernel(hidden_state, encoder_outputs, W1, b1, w2, _trace=False, _trace_kwargs=None):
    hidden_state = np.asarray(hidden_state, dtype=np.float32)
    encoder_outputs = np.asarray(encoder_outputs, dtype=np.float32)
    W1 = np.asarray(W1, dtype=np.float32)
    b1 = np.asarray(b1, dtype=np.float32)
    w2 = np.asarray(w2, dtype=np.float32)

    nc = _get_nc()
    in_maps = [
        _prep_core_inputs(hidden_state, encoder_outputs, W1, b1, w2, c)
        for c in range(NCORES)
    ]
    res = run_bass_kernel_spmd(
        nc, in_maps, list(range(NCORES)), trace=_trace,
        **(_trace_kwargs or {}),
    )
    out = np.empty((B, D), dtype=np.float32)
    for c in range(NCORES):
        r = res.results[c]["ctx"]                          # [p, di, b]
        out[c * BPC:(c + 1) * BPC] = r.transpose(2, 1, 0).reshape(BPC, D)
    if _trace:
        return out, res
    return out


# revision 11
# speedup vs baseline: 1.0564x; 1.0223x over previous
"""Bahdanau additive attention kernel for Trainium2 (8 NeuronCores).

Reference computation (B=32, S=4096, D=512):
    pre   = enc @ We.T + (hidden @ Wh.T + b1)[:, None, :]   # [B, S, D]
    h     = tanh(pre)
    e     = h @ w2                                          # [B, S]
    alpha = softmax(e, axis=1)
    ctx   = einsum('bs,bsd->bd', alpha, enc)                # [B, D]

Data-parallel over batch (4 batches per core); enc is re-laid-out on host
as [b, di, p, s] bf16 so the contraction dim d sits on SBUF partitions.

Precision plan (gate is rel-l2 < 2e-2 on ctx):
  - ctx is a softmax-weighted mean of zero-mean vectors - the signal
    shrinks exactly as fast as independent noise averages out, so any
    per-element quantization error lands on ctx at full relative
    strength. Full-fp8 main matmul measures 2.2e-2 (fails); half the
    contraction in fp8 measures ~1.6e-2 (passes).
  - So the main matmul contracts d 0..255 with ONE fp8-e4m3 DoubleRow
    matmul per (ki, s-block) (256-deep contraction per instruction,
    ~0.57x the PE cost of bf16) and d 256..511 with 2 bf16 matmuls.
  - Scale bridge: fp8 operands are scaled (enc x32, W x1024) so their
    PSUM contribution is 2^15 x pre; the bf16 weight half is pre-scaled
    x2^15 on host (exact power of two), and the tanh ACT applies
    scale=2^-15. All contributions share one PSUM accumulation group
    with zero extra ops.
  - Everything downstream of pre stays bf16 (h, p rows, ctx-accum enc).

Schedule (per core):
  - The tiny bias vector c = hidden @ Wh.T + b1 is computed on host
    and rides the ACT bias operand.
  - PE warm-up burst on a memzero'd tile brings the HAM clock gate to
    8/8 while the first enc tile streams in.
  - e matmuls are column-group packed (tile_position=(0,0)/(0,32)) into
    ONE [48, 512] PSUM tile: group 0 uses M=32 (w2 in column 0, zeros
    elsewhere) so PSUM rows 1..31 are written zeros, letting a SINGLE
    exp ACT over partitions [0:33] handle both groups' rows (halves the
    Scalar exp cost vs two [1,512] ACTs; rows 1..31 accumulate exp(0)
    into lp rows that the finalize never reads).
  - Half-tile software pipelining: tile t's e/softmax/context chain is
    emitted after tile t+1's first ki group, so the in-order PE queue
    never waits on tanh.
  - exp runs with fused row-sum accumulation (unnormalized streaming
    softmax: |e| < ~5 so no max pass); per-batch row sums land in lp
    rows 0/32, are reduced along tiles, row 32 is staged down to
    partition 0 by a tiny DMA, summed, reciprocal'd, and broadcast to
    [128,1] for the final context scaling.
  - alpha rows are replicated across partitions by gpsimd
    partition_broadcast COMPUTE ops (no DRAM round trip; broadcast APs
    must be canonical - full-tile dest, partition-0/offset-0 source, so
    the second e row is staged down from partition 32 by a tiny
    scalar-issued copy; sliced/offset APs silently misread on HW).
  - Strict DMA queue roles: sync carries ONLY enc tiles (fp8 pairs tile
    first, then bf16), so the PE-feeding stream is never queued behind
    softmax-dependent writes; gpsimd issues NO DMAs at all.
  - Variable s-tile widths: batch 0 opens with a 512-wide tile (halves
    the pipeline-fill DMA latency), the last batch closes with two
    512-wide tiles (halves the post-PE softmax/context drain).
"""

import sys

if "/opt/trn_rl_repo" not in sys.path:
    sys.path.insert(0, "/opt/trn_rl_repo")

from contextlib import ExitStack

import ml_dtypes
import numpy as np

import concourse.bass as bass
import concourse.bacc as bacc
from concourse import bass_isa
import concourse.tile as tile
from concourse import mybir
from concourse.bass_utils import run_bass_kernel_spmd

B, S, D = 32, 4096, 512
NCORES = 8
BPC = B // NCORES          # batches per core
P = 128                    # partitions
NDC = D // P               # d (contraction) chunks
NKC = D // P               # k (output channel) chunks
ST = 1024                  # max s-tile width (buffer sizing)
EST = 512                  # e-row granularity
SE = 32.0                  # fp8 enc scale
SW = 1024.0                # fp8 weight scale
SINV = 1.0 / (SE * SW)     # 2^-15, ACT tanh scale
# s-tile widths per batch: a narrow first tile on batch 0 shortens the
# pipeline-fill DMA latency; narrow last tiles on the final batch shorten
# the softmax/context drain after the PE finishes.
BATCH_WIDTHS = [
    [512, 1024, 1024, 1024, 512],
    [1024, 1024, 1024, 1024],
    [1024, 1024, 1024, 1024],
    [1024, 1024, 1024, 512, 256, 256],
]
NST = max(len(w) for w in BATCH_WIDTHS)

F32 = mybir.dt.float32
BF16 = mybir.dt.bfloat16
FP8 = mybir.dt.float8e4
AF = mybir.ActivationFunctionType
ALU = mybir.AluOpType
DR = mybir.MatmulPerfMode.DoubleRow


def build_bass():
    nc = bacc.Bacc()

    encT = nc.declare_dram_parameter("encT", [BPC, NDC, P, S], BF16, isOutput=False)
    enc8T = nc.declare_dram_parameter("enc8T", [BPC, P, 2, S], FP8, isOutput=False)
    # weT[i, p, k] = We[k, 256 + i*128 + p] * 2^15  (bf16 half, pre-scaled)
    weT = nc.declare_dram_parameter("weT", [2, P, D], BF16, isOutput=False)
    # w8T[p, j, k] = We[k, j*128 + p] * 1024  (fp8 DoubleRow half)
    w8T = nc.declare_dram_parameter("w8T", [P, 2, D], FP8, isOutput=False)
    cT = nc.declare_dram_parameter("cT", [P, NKC, BPC], F32, isOutput=False)
    # w2c[p, ki, m]: column 0 holds w2[ki*128+p], columns 1..31 zero
    w2c = nc.declare_dram_parameter("w2c", [P, NKC, 32], BF16, isOutput=False)
    ctx_out = nc.declare_dram_parameter("ctx", [P, NDC, BPC], F32, isOutput=True)

    with TileKernel(nc) as tk:
        tk.build(encT, enc8T, weT, w8T, cT, w2c, ctx_out)
    nc.finalize()
    return nc


class TileKernel:
    def __init__(self, nc):
        self.nc = nc
        self.stack = ExitStack()
        self.tc = None

    def __enter__(self):
        self.tc = self.stack.enter_context(tile.TileContext(self.nc))
        return self

    def __exit__(self, *exc):
        return self.stack.__exit__(*exc)

    def build(self, encT, enc8T, weT, w8T, cT, w2c, ctx_out):
        nc, tc, ctx = self.nc, self.tc, self.stack

        singles = ctx.enter_context(tc.tile_pool(name="singles", bufs=1))
        encp = ctx.enter_context(tc.tile_pool(name="encp", bufs=8))
        encp8 = ctx.enter_context(tc.tile_pool(name="encp8", bufs=8))
        htp = ctx.enter_context(tc.tile_pool(name="htp", bufs=4))
        abp = ctx.enter_context(tc.tile_pool(name="abp", bufs=8))
        junkp = ctx.enter_context(tc.tile_pool(name="junkp", bufs=3))
        smp = ctx.enter_context(tc.tile_pool(name="smp", bufs=3))
        ctxp = ctx.enter_context(tc.tile_pool(name="ctxp", bufs=2))
        prep = ctx.enter_context(tc.tile_pool(name="prep", bufs=3, space="PSUM"))
        ecp = ctx.enter_context(tc.tile_pool(name="ecp", bufs=2, space="PSUM"))

        # ---- load constants ----
        # constants go via the scalar queue so they don't delay enc tile 0
        # on the sync queue.
        c_sb = singles.tile([P, NKC, BPC], F32)
        nc.scalar.dma_start(out=c_sb, in_=cT[:])
        w8_sb = singles.tile([P, 2, D], FP8)
        nc.scalar.dma_start(out=w8_sb, in_=w8T[:])
        w_sb = singles.tile([P, 2, D], BF16)
        nc.scalar.dma_start(out=w_sb, in_=weT[:].rearrange("i p k -> p i k"))
        w2_sb = singles.tile([P, NKC, 32], BF16)
        nc.scalar.dma_start(out=w2_sb, in_=w2c[:])
        # ---- PE warm-up burst ----
        # ~2.3 us of dummy matmuls on a zeroed tile (no DMA dependency) so
        # the HAM clock gate reaches 8/8 before real work starts.
        wz = singles.tile([P, D], BF16)
        nc.vector.memzero(wz)
        wpre = prep.tile([P, ST], F32, tag="pre")
        for i in range(11):
            nc.tensor.matmul(
                out=wpre[:, 0:D], lhsT=wz[:, 0:P], rhs=wz,
                start=True, stop=True,
            )
        wjunk = singles.tile([P, 1], F32)
        nc.vector.tensor_copy(out=wjunk, in_=wpre[:, 0:1])

        # ---- main per-batch pipeline ----
        # `prev` carries one pending softmax/context chain ACROSS batch
        # boundaries (flushed inside the next tile's first ki group), so
        # the PE never drains at a batch edge; each chain closes over its
        # own batch's lp/cacc tiles.
        prev = None
        pend_fin = None

        def softmax_ctx(t, w, et, ht, lp, cacc, is_last):
            ng = (w + EST - 1) // EST
                # e rows, packed on PE column groups 0/32 into one PSUM
                # tile: the (up to) two accumulation chains run
                # concurrently (separate XBUS streams, separate PSUM
                # partition ranges). Group 0 uses M=32 so rows 1..31 are
                # written zeros - that makes the single [0:33] exp ACT
                # read only initialized PSUM.
            # The two chains are separate accumulation groups over
            # disjoint partition ranges of one tile; pending-zero is
            # tracked per out-AP partitions, so each chain needs its
            # own start. skip_group_check silences the coarser
            # tile-framework zero-region conflict check for chain 1.
            w0 = min(w, EST)
            w1 = w - w0
            e_ps = ecp.tile([48, EST], F32, tag="ec")
            for ki in range(NKC):
                nc.tensor.matmul(
                    out=e_ps[0:32, 0:w0],
                    lhsT=w2_sb[:, ki, :],
                    rhs=ht[:, ki, 0:w0],
                    start=(ki == 0),
                    stop=(ki == NKC - 1),
                    tile_position=(0, 0),
                )
                if ng == 2:
                    nc.tensor.matmul(
                        out=e_ps[32:48, 0:w1],
                        lhsT=w2_sb[:, ki, 0:16],
                        rhs=ht[:, ki, EST:EST + w1],
                        start=(ki == 0),
                        stop=(ki == NKC - 1),
                        tile_position=(0, 32),
                        skip_group_check=True,
                    )
            # p = exp(e) with the row-sum fused; one ACT covers both
            # groups' live rows (partitions 0 and 32). Merging requires
            # both groups full-width.
            p_rows = smp.tile([48, EST], BF16, tag="prow")
            np_act = 33 if (ng == 2 and w1 == EST) else 1
            nc.scalar.activation(
                out=p_rows[0:np_act, 0:w0],
                in_=e_ps[0:np_act, 0:w0],
                func=AF.Exp, bias=0.0, scale=1.0,
                accum_out=lp[0:np_act, t:t + 1],
            )
            if ng == 2 and np_act == 1:
                nc.scalar.activation(
                    out=p_rows[32:33, 0:w1],
                    in_=e_ps[32:33, 0:w1],
                    func=AF.Exp, bias=0.0, scale=1.0,
                    accum_out=lp[32:33, t:t + 1],
                )
            if is_last:
                # early lsum extraction: the exp row sums are complete
                # once this (final) chain's exp has run; start the
                # cross-partition staging DMA now so its latency hides
                # under the context accumulation below.
                lpr = smp.tile([48, 1], F32, tag="lpr")
                nc.vector.reduce_sum(out=lpr, in_=lp, axis=mybir.AxisListType.X)
                ls1 = smp.tile([1, 1], F32, tag="ls1")
                nc.sync.dma_start(out=ls1, in_=lpr[32:33, :])
                self._fin = (lpr, ls1)
            # replicate the p rows across partitions with gpsimd
            # partition_broadcast compute ops: no DRAM round trip, and
            # gpsimd stays DMA-free. The broadcast gets canonical APs
            # only: full-tile outputs, and a partition-0/offset-0
            # source (row g=1 is staged down from partition 32 by a
            # tiny scalar-issued SBUF->SBUF copy).
            ab = abp.tile([P, ST], BF16, tag="ab")
            for g in range(ng):
                if g == 0:
                    src_row = p_rows[0:1, 0:w0]
                    dst = ab[:, 0:w0]
                else:
                    pr1 = smp.tile([1, EST], BF16, tag="pr1")
                    nc.sync.dma_start(out=pr1[:, 0:w1], in_=p_rows[32:33, 0:w1])
                    src_row = pr1[:, 0:w1]
                    dst = ab[:, EST:EST + w1]
                nc.gpsimd.partition_broadcast(out_ap=dst, in_ap=src_row)
            for di in (2, 3, 0, 1):
                junk = junkp.tile([P, ST], BF16, tag="junk")
                nc.vector.scalar_tensor_tensor(
                    out=junk[:, 0:w],
                    in0=et[:, di, 0:w],
                    scalar=1.0,
                    in1=ab[:, 0:w],
                    op0=ALU.mult,
                    op1=ALU.mult,
                    accum_out=cacc[:, di, t:t + 1],
                )

        def finalize(b, lp, cacc):
            # ---- finalize: ctx = (sum_s p*enc) / sum_s p ----
            # lp rows 0 and 32 hold the two groups' per-tile exp sums
            # (rows 1..31 hold junk exp(0) sums the finalize never
            # reads); the reduce + row-32 staging DMA were emitted with
            # the batch's last chain.
            lpr, ls1 = self._fin
            lsum = smp.tile([1, 1], F32, tag="lsum")
            nc.vector.tensor_add(out=lsum, in0=lpr[0:1, :], in1=ls1)
            rinv1 = smp.tile([1, 1], F32, tag="rinv1")
            nc.vector.reciprocal(out=rinv1, in_=lsum)
            rinvb = smp.tile([P, 1], F32, tag="rinvb")
            nc.gpsimd.partition_broadcast(out_ap=rinvb, in_ap=rinv1)
            ctx_acc = ctxp.tile([P, NDC], F32, tag="ctx")
            nc.vector.reduce_sum(out=ctx_acc, in_=cacc, axis=mybir.AxisListType.X)
            nc.vector.tensor_scalar_mul(out=ctx_acc, in0=ctx_acc, scalar1=rinvb)
            nc.sync.dma_start(out=ctx_out[:][:, :, b], in_=ctx_acc)

        for b in range(BPC):
            nbt = len(BATCH_WIDTHS[b])
            lp = smp.tile([48, NST], F32, tag="lp")
            nc.vector.memzero(lp)
            cacc = ctxp.tile([P, NDC, nbt], F32, tag="cacc")

            s0 = 0
            for t, w in enumerate(BATCH_WIDTHS[b]):
                et8 = encp8.tile([P, 2, ST], FP8, tag="et8")
                nc.sync.dma_start(
                    out=et8[:, :, 0:w],
                    in_=enc8T[:][b, :, :, s0:s0 + w])
                et = encp.tile([P, NDC, ST], BF16, tag="et")
                nc.sync.dma_start(
                    out=et[:, :, 0:w],
                    in_=encT[:][b, :, :, s0:s0 + w].rearrange("di p s -> p di s"))

                ht = htp.tile([P, NKC, ST], BF16, tag="ht")
                shift_tanh = (b == 0 and t == 0)
                pend_tanh = []
                for ki in range(NKC):
                    pre_ps = prep.tile([P, ST], F32, tag="pre")
                    for half in range((w + EST - 1) // EST):
                        hw_ = min(EST, w - half * EST)
                        sl = slice(half * EST, half * EST + hw_)
                        # d 0..255: one fp8 DoubleRow matmul (256-deep)
                        nc.tensor.matmul(
                            out=pre_ps[:, sl],
                            lhsT=w8_sb[:, :, ki * P:(ki + 1) * P],
                            rhs=et8[:, :, sl],
                            start=True,
                            stop=False,
                            perf_mode=DR,
                        )
                        # d 256..511: two bf16 matmuls (weights x 2^15)
                        for i in range(2):
                            nc.tensor.matmul(
                                out=pre_ps[:, sl],
                                lhsT=w_sb[:, i, ki * P:(ki + 1) * P],
                                rhs=et[:, 2 + i, sl],
                                start=False,
                                stop=(i == 1),
                            )
                    # h^T = tanh(2^-15 * pre^T + c), one ACT op per ki.
                    # On the first tile the emission is shifted one ki
                    # later so the pipe-fill matmuls never wait on it.
                    def emit_tanh(ki, pre_ps):
                        nc.scalar.activation(
                            out=ht[:, ki, 0:w],
                            in_=pre_ps[:, 0:w],
                            func=AF.Tanh,
                            bias=c_sb[:, ki, b:b + 1],
                            scale=SINV,
                        )
                    if shift_tanh:
                        pend_tanh.append((ki, pre_ps))
                        if len(pend_tanh) > 1:
                            emit_tanh(*pend_tanh.pop(0))
                    else:
                        emit_tanh(ki, pre_ps)
                    # half-tile software pipelining: the previous tile's
                    # e/softmax/context chain slots in after this tile's
                    # first ki group, when its tanh inputs are ready but
                    # well before the PE would stall on them. Chains (and
                    # the previous batch's finalize) cross batch edges.
                    if ki == 1 and prev is not None:
                        softmax_ctx(*prev)
                        prev = None
                        if pend_fin is not None:
                            finalize(*pend_fin)
                            pend_fin = None
                for args_ in pend_tanh:
                    emit_tanh(*args_)
                prev = (t, w, et, ht, lp, cacc, False)
                s0 += w
            prev = prev[:6] + (True,)
            pend_fin = (b, lp, cacc)

        softmax_ctx(*prev)
        finalize(*pend_fin)


_NC_CACHE = None


def _get_nc():
    global _NC_CACHE
    if _NC_CACHE is None:
        _NC_CACHE = build_bass()
    return _NC_CACHE


def _prep_core_inputs(hidden_state, encoder_outputs, W1, b1, w2, core):
    bf16 = ml_dtypes.bfloat16
    e4m3 = ml_dtypes.float8_e4m3
    b0 = core * BPC
    enc = encoder_outputs[b0:b0 + BPC]                      # [BPC, S, D] f32
    # [b, d, s] -> [b, di, p, s] flat along s
    e = enc.transpose(0, 2, 1).reshape(BPC, NDC, P, S).astype(bf16)
    e = np.ascontiguousarray(e)
    # fp8 pairs for d 0..255: enc8[b, p, j, s] = e4m3(32 * enc[b, s, j*128+p])
    e8 = np.clip(enc.transpose(0, 2, 1)[:, 0:256, :] * SE, -240.0, 240.0)
    e8 = np.ascontiguousarray(
        e8.reshape(BPC, 2, P, S).transpose(0, 2, 1, 3)).astype(e4m3)
    # bf16 weight half (d 256..511), pre-scaled by 2^15
    weS = (W1[:, 256:D].T * (SE * SW)).reshape(2, P, D)
    # fp8 weight half (d 0..255): w8[p, j, k] = e4m3(1024 * We[k, j*128+p])
    w8 = np.clip(W1[:, 0:256].T * SW, -240.0, 240.0).reshape(2, P, D)
    w8 = np.ascontiguousarray(w8.transpose(1, 0, 2)).astype(e4m3)
    w2cv = np.zeros((P, NKC, 32), dtype=np.float32)
    w2cv[:, :, 0] = w2.reshape(NKC, P).T
    return {
        "encT": e,
        "enc8T": e8,
        "weT": np.ascontiguousarray(weS).astype(bf16),
        "w8T": w8,
        "cT": np.ascontiguousarray(
            (hidden_state[b0:b0 + BPC] @ W1[:, D:].T + b1).T.reshape(NKC, P, BPC)
            .transpose(1, 0, 2)),
        "w2c": w2cv.astype(bf16),
    }


def kernel(hidden_state, encoder_outputs, W1, b1, w2, _trace=False, _trace_kwargs=None):
    hidden_state = np.asarray(hidden_state, dtype=np.float32)
    encoder_outputs = np.asarray(encoder_outputs, dtype=np.float32)
    W1 = np.asarray(W1, dtype=np.float32)
    b1 = np.asarray(b1, dtype=np.float32)
    w2 = np.asarray(w2, dtype=np.float32)

    nc = _get_nc()
    in_maps = [
        _prep_core_inputs(hidden_state, encoder_outputs, W1, b1, w2, c)
        for c in range(NCORES)
    ]
    res = run_bass_kernel_spmd(
        nc, in_maps, list(range(NCORES)), trace=_trace,
        **(_trace_kwargs or {}),
    )
    out = np.empty((B, D), dtype=np.float32)
    for c in range(NCORES):
        r = res.results[c]["ctx"]                          # [p, di, b]
        out[c * BPC:(c + 1) * BPC] = r.transpose(2, 1, 0).reshape(BPC, D)
    if _trace:
        return out, res
    return out
